# revision 3
# baseline (speedup 1.0000x reference)
"""Trainium2 Bass kernel for the pairwise-score attention + gated MLP encoding.

Computation (per batch element b, p=1024 tokens, d=256 features):
    A[i,j]  = wa.P_i + wb.P_j + (P_i*wc).P_j
    itr     = softmax_j(A) @ P
    cat     = [P, itr]
    z       = tanh(cat@w1+b1); r = sigmoid(cat@w2+b2); f = sigmoid(cat@w3+b3)
    out     = r*P + f*z
Sharding: data-parallel over batch across 8 NeuronCores (4 batch el / core).

v2 design (fp8-DoubleRow scores/attention, bf16 MLP):
  - P loaded twice via gpsimd casting DMAs: natural bf16 (Pnb) and natural
    fp8e4 (Pn8).  P^T (bf16, PTb) via 16 XBAR DMA-transposes per batch on the
    sync/scalar HWDGE queues -- zero PE cost.
  - Scores^T[j,i] (wa-term cancels in softmax; wb-term folded into the exp
    bias) via fp8e4 DoubleRow matmuls (contract 256/pass, 0.5 cyc/row):
    stationary PT8[:, :, jc-slice], moving PcT8 = wc*PTb (DVE).
  - exp on ACT straight from 2-bank PSUM, fp8e4 output, sb bias.
  - Softmax denominator (ones8 stationary) and itr^T numerator (Pn8 pair
    stationary) also fp8 DoubleRow over expST8.  DVE normalizes with
    reciprocal + mul into bf16 itrT.
  - MLP transposed in bf16 (out^T = (cat@w)^T), biases as ACT per-partition
    biases; sigmoid via 0.5+0.5*tanh(x/2) (single ACT table set).
  - Gating computes o' = 2*out in bf16 (3 DVE ops/chunk); the 0.5 rides the
    PSUM-evacuation tensor_scalar_mul after the bf16 PE output transposes.
  - Software-pipelined across batch elements so PE/ACT/DVE/GpSimd all stay
    busy; helper work is spread: loads+weights on gpsimd sw-DGE, transposes
    on the DMA XBAR, evacuations/casts split DVE-side.
"""

import os
import sys

if "/opt/trn_rl_repo" not in sys.path:
    sys.path.insert(0, "/opt/trn_rl_repo")

import numpy as np

import concourse.bass as bass
import concourse.mybir as mybir
import concourse.tile as tile
from concourse import bacc
from concourse.bass_utils import run_bass_kernel_spmd
from concourse.masks import make_identity

F32 = mybir.dt.float32
BF16 = mybir.dt.bfloat16
FP8 = mybir.dt.float8e4
AF = mybir.ActivationFunctionType
ALU = mybir.AluOpType
AXX = mybir.AxisListType
DR = mybir.MatmulPerfMode.DoubleRow

B, PLEN, D = 32, 1024, 256
N_CORES = 8
B_LOC = B // N_CORES  # batch elements per core

NJ = PLEN // 128  # 8 token chunks of 128
ND = D // 128     # 2 feature chunks of 128
NPAIR = NJ // 2   # 4 token chunk-pairs (DoubleRow contraction)


def _emit(ctx, tc, P_in, w_att, w_mlp, b_mlp, out):
    nc = tc.nc
    ts = bass.ts
    ds = bass.ds

    const = ctx.enter_context(tc.tile_pool(name="const", bufs=1))
    pload = ctx.enter_context(tc.tile_pool(name="pload", bufs=3))
    ptp = ctx.enter_context(tc.tile_pool(name="ptp", bufs=2))
    pexp = ctx.enter_context(tc.tile_pool(name="pexp", bufs=2))
    pitr = ctx.enter_context(tc.tile_pool(name="pitr", bufs=2))
    pmlp = ctx.enter_context(tc.tile_pool(name="pmlp", bufs=2))
    pout = ctx.enter_context(tc.tile_pool(name="pout", bufs=2))
    ps_big = ctx.enter_context(tc.tile_pool(name="ps_big", bufs=3, space="PSUM"))
    ps_t2 = ctx.enter_context(tc.tile_pool(name="ps_t2", bufs=2, space="PSUM"))

    # ---- batch loads (emitted first so DMA starts immediately) ----
    def phase_load(b):
        pnb = pload.tile([128, NJ, 256], BF16, tag="pnb", name=f"pnb{b}")
        pn8 = pload.tile([128, NJ, 256], FP8, tag="pn8", name=f"pn8{b}")
        src = P_in[b, :, :].rearrange("(jc p) d -> p jc d", p=128)
        nc.gpsimd.dma_start(out=pnb, in_=src)
        nc.gpsimd.dma_start(out=pn8, in_=src)
        return pnb, pn8

    ld0 = phase_load(0)

    # ---- constants / weights (once per core) ----
    ident = const.tile([128, 128], F32)
    make_identity(nc, ident)
    identb = const.tile([128, 128], BF16)
    nc.vector.tensor_copy(out=identb, in_=ident)
    ones_f = const.tile([128, 256], F32)
    nc.vector.memset(ones_f, 1.0)
    ones8 = const.tile([128, 2, 128], FP8)
    nc.vector.tensor_copy(out=ones8, in_=ones_f.rearrange("p (t m) -> p t m", t=2))

    # wc per-partition scalars in transposed layout: [128,1] per d-chunk
    wc_sb = []
    for dc in range(ND):
        wc = const.tile([128, 1], F32, tag=f"wc{dc}")
        nc.gpsimd.dma_start(out=wc,
                            in_=w_att[ds(2 * D + dc * 128, 128)].unsqueeze(1))
        wc_sb.append(wc)
    # wb broadcast to all partitions and repeated per token-chunk:
    # [128, 8, 256] bf16 (for the sb row-reduction against natural-layout P)
    _wbs = w_att[ds(D, D)]
    wbb8 = const.tile([128, NJ, 256], BF16)
    nc.gpsimd.dma_start(
        out=wbb8,
        in_=bass.AP(tensor=_wbs.tensor, offset=_wbs.offset,
                    ap=[[0, 128], [0, NJ]] + list(_wbs.ap)),
    )

    # MLP weights: [512, 256] -> sbuf [128, 4(kc), 256] bf16 via casting DMA
    w_sb = []
    for wi in range(3):
        wt = const.tile([128, 4, D], BF16, tag=f"w{wi}")
        nc.gpsimd.dma_start(
            out=wt, in_=w_mlp[wi].rearrange("(kc k) d -> k kc d", k=128))
        w_sb.append(wt)

    # biases, per dout-chunk [128,1]; for r/f (sigmoid-via-tanh) we need b/2
    b_sb = []  # b_sb[wi][dc]
    for wi in range(3):
        chunks = []
        for dc in range(ND):
            bt = const.tile([128, 1], F32, tag=f"b{wi}{dc}")
            nc.gpsimd.dma_start(out=bt,
                                in_=b_mlp[wi][ds(dc * 128, 128)].unsqueeze(1))
            if wi > 0:
                bh = const.tile([128, 1], F32, tag=f"bh{wi}{dc}")
                nc.scalar.mul(out=bh, in_=bt, mul=0.5)
                bt = bh
            chunks.append(bt)
        b_sb.append(chunks)

    # ---- per-batch-element phases ----
    def phase_xbar(b, pnb):
        # P^T (bf16) via XBAR DMA transposes: zero PE cost
        ptb = ptp.tile([128, ND, PLEN], BF16, tag="ptb", name=f"ptb{b}")
        for jc in range(NJ):
            eng = nc.sync if jc % 2 == 0 else nc.scalar
            for dc in range(ND):
                eng.dma_start(out=ptb[:, dc, ts(jc, 128)],
                              in_=pnb[:, jc, ts(dc, 128)], transpose=True)
        return ptb

    def phase_prep(b, pnb, ptb):
        pt8 = ptp.tile([128, ND, PLEN], FP8, tag="pt8", name=f"pt8{b}")
        nc.vector.tensor_copy(out=pt8, in_=ptb)
        pct8 = ptp.tile([128, ND, PLEN], FP8, tag="pct8", name=f"pct8{b}")
        for dc in range(ND):
            nc.vector.tensor_scalar_mul(out=pct8[:, dc, :], in0=ptb[:, dc, :],
                                        scalar1=wc_sb[dc])
        # sb[j] = P_j . wb : one big mul + one row-reduce
        scr = ptp.tile([128, NJ, 256], BF16, tag="scr", name=f"scr{b}")
        nc.vector.tensor_mul(out=scr, in0=pnb, in1=wbb8)
        sbc = ptp.tile([128, NJ], F32, tag="sbc", name=f"sbc{b}")
        nc.vector.reduce_sum(out=sbc, in_=scr, axis=AXX.X)
        return pt8, pct8, sbc

    def phase_scores(b, pt8, pct8, sbc):
        es = [pexp.tile([128, 2, PLEN], FP8, tag=f"es{pr}", name=f"es{pr}_{b}")
              for pr in range(NPAIR)]
        for jc in range(NJ):
            pss = ps_big.tile([128, 1024], F32, tag="big", name=f"pss{jc}")
            for ic2 in range(2):
                nc.tensor.matmul(pss[:, ts(ic2, 512)],
                                 pt8[:, :, ts(jc, 128)],
                                 pct8[:, :, ts(ic2, 512)],
                                 start=True, stop=True, perf_mode=DR)
            nc.scalar.activation(out=es[jc // 2][:, jc % 2, :], in_=pss,
                                 func=AF.Exp, bias=sbc[:, ds(jc, 1)], scale=1.0)
        return es

    def phase_attn(b, pn8, es):
        # softmax denominators (replicated across partitions by ones8)
        psd = ps_big.tile([128, 1024], F32, tag="big", name="psd")
        for ic2 in range(2):
            for pr in range(NPAIR):
                nc.tensor.matmul(psd[:, ts(ic2, 512)], ones8,
                                 es[pr][:, :, ts(ic2, 512)],
                                 start=(pr == 0), stop=(pr == NPAIR - 1),
                                 perf_mode=DR)
        # itr^T numerator, directly in the layout the MLP consumes
        pits = []
        for dc in range(ND):
            pit = ps_big.tile([128, 1024], F32, tag="big", name=f"pit{dc}")
            for ic2 in range(2):
                for pr in range(NPAIR):
                    nc.tensor.matmul(pit[:, ts(ic2, 512)],
                                     pn8[:, ds(2 * pr, 2), ts(dc, 128)],
                                     es[pr][:, :, ts(ic2, 512)],
                                     start=(pr == 0), stop=(pr == NPAIR - 1),
                                     perf_mode=DR)
            pits.append(pit)
        return psd, pits

    def phase_norm(b, psd, pits):
        recipb = pitr.tile([128, PLEN], F32, tag="recipb", name=f"rec{b}")
        nc.vector.reciprocal_approx_fast(out=recipb, in_=psd)
        itrT = []
        for dc in range(ND):
            it = pitr.tile([128, PLEN], BF16, tag=f"it{dc}", name=f"itrT{dc}_{b}")
            nc.vector.tensor_mul(out=it, in0=pits[dc], in1=recipb)
            itrT.append(it)
        return itrT

    def phase_mlp(b, ptb, itrT):
        # moving operands for the 4 contraction chunks of cat^T
        catT = [ptb[:, 0, :], ptb[:, 1, :], itrT[0], itrT[1]]
        acts = []  # acts[dc][wi]
        for dc in range(ND):
            row = []
            for wi in range(3):
                psm = ps_big.tile([128, 1024], F32, tag="big", name=f"psm{dc}{wi}")
                for pc in range(2):
                    for kc in range(4):
                        nc.tensor.matmul(
                            psm[:, ts(pc, 512)],
                            w_sb[wi][:, kc, ts(dc, 128)],
                            catT[kc][:, ts(pc, 512)],
                            start=(kc == 0), stop=(kc == 3),
                        )
                t = pmlp.tile([128, PLEN], BF16, tag=f"act{wi}", name=f"a{wi}d{dc}")
                if wi == 0:
                    nc.scalar.activation(out=t, in_=psm, func=AF.Tanh,
                                         bias=b_sb[0][dc], scale=1.0)
                else:
                    nc.scalar.activation(out=t, in_=psm, func=AF.Tanh,
                                         bias=b_sb[wi][dc], scale=0.5)
                row.append(t)
            acts.append(row)
        return acts

    def phase_gate(b, ptb, acts):
        # o' = 2*out = (t2+1)*P + (t3+1)*z ; the 0.5 rides the out-evacuation
        oT = []
        for dc in range(ND):
            z_t, t2, t3 = acts[dc]
            o = pmlp.tile([128, PLEN], BF16, tag=f"oT{dc}", name=f"oT{dc}_{b}")
            for pc in range(2):
                sl = ts(pc, 512)
                m1 = pmlp.tile([128, 512], BF16, tag="m1", name="m1", bufs=2)
                nc.vector.scalar_tensor_tensor(out=m1, in0=t2[:, sl], scalar=1.0,
                                               in1=ptb[:, dc, sl],
                                               op0=ALU.add, op1=ALU.mult)
                m2 = pmlp.tile([128, 512], BF16, tag="m2", name="m2", bufs=2)
                nc.vector.scalar_tensor_tensor(out=m2, in0=t3[:, sl], scalar=1.0,
                                               in1=z_t[:, sl],
                                               op0=ALU.add, op1=ALU.mult)
                nc.vector.scalar_tensor_tensor(out=o[:, sl], in0=m1, scalar=1.0,
                                               in1=m2, op0=ALU.mult,
                                               op1=ALU.add)
            oT.append(o)
        return oT

    def phase_out(b, oT):
        for p2 in range(NJ):
            pst = ps_t2.tile([128, 256], BF16, tag="pst", name="pst")
            nc.tensor.transpose(pst[:, 0:128], oT[0][:, ts(p2, 128)], identb)
            nc.tensor.transpose(pst[:, 128:256], oT[1][:, ts(p2, 128)], identb)
            onat = pout.tile([128, D], F32, tag=f"on{p2}", name=f"onat{p2}")
            nc.vector.tensor_scalar_mul(out=onat, in0=pst, scalar1=0.5)
            nc.sync.dma_start(out=out[b, ts(p2, 128), :], in_=onat)

    # ---- software-pipelined emission across batch elements ----
    lds = {0: ld0, 1: phase_load(1)}
    ptb0 = phase_xbar(0, lds[0][0])
    pt8_0, pct8_0, sbc0 = phase_prep(0, lds[0][0], ptb0)
    es = phase_scores(0, pt8_0, pct8_0, sbc0)
    cur = {"pnb": lds[0][0], "pn8": lds[0][1], "ptb": ptb0}
    nxt = {}
    oT_prev = None
    ptb_prev = None
    for b in range(B_LOC):
        if b + 2 < B_LOC:
            lds[b + 2] = phase_load(b + 2)
        if b + 1 < B_LOC:
            nxt = {"pnb": lds[b + 1][0], "pn8": lds[b + 1][1]}
            nxt["ptb"] = phase_xbar(b + 1, nxt["pnb"])
        psd, pits = phase_attn(b, cur["pn8"], es)
        itrT = phase_norm(b, psd, pits)
        if oT_prev is not None:
            phase_out(b - 1, oT_prev)
        if b + 1 < B_LOC:
            pt8_n, pct8_n, sbc_n = phase_prep(b + 1, nxt["pnb"], nxt["ptb"])
        acts = phase_mlp(b, cur["ptb"], itrT)
        oT = phase_gate(b, cur["ptb"], acts)
        if b + 1 < B_LOC:
            es = phase_scores(b + 1, pt8_n, pct8_n, sbc_n)
        oT_prev = oT
        if b + 1 < B_LOC:
            cur = nxt
    phase_out(B_LOC - 1, oT_prev)


_NC_CACHE = {}


def _build():
    if "nc" in _NC_CACHE:
        return _NC_CACHE["nc"]
    nc = bacc.Bacc("TRN2", target_bir_lowering=False, debug=False,
                   num_devices=N_CORES)
    P_in = nc.dram_tensor("p_in", [B_LOC, PLEN, D], F32, kind="ExternalInput").ap()
    w_att = nc.dram_tensor("w_att", [3 * D], F32, kind="ExternalInput").ap()
    w_mlp = [nc.dram_tensor(f"w{i}", [2 * D, D], F32, kind="ExternalInput").ap()
             for i in (1, 2, 3)]
    b_mlp = [nc.dram_tensor(f"b{i}", [D], F32, kind="ExternalInput").ap()
             for i in (1, 2, 3)]
    out = nc.dram_tensor("out", [B_LOC, PLEN, D], F32, kind="ExternalOutput").ap()

    from contextlib import ExitStack

    with tile.TileContext(nc) as tc, ExitStack() as ctx:
        _emit(ctx, tc, P_in, w_att, w_mlp, b_mlp, out)
    nc.compile()
    _NC_CACHE["nc"] = nc
    return nc


def run(inputs, trace=False, tmpdir=None):
    nc = _build()
    P = np.ascontiguousarray(np.asarray(inputs["P"], dtype=np.float32))
    shared = {
        "w_att": np.ascontiguousarray(np.asarray(inputs["w_itr_att"], np.float32)),
        "w1": np.ascontiguousarray(np.asarray(inputs["w1"], np.float32)),
        "w2": np.ascontiguousarray(np.asarray(inputs["w2"], np.float32)),
        "w3": np.ascontiguousarray(np.asarray(inputs["w3"], np.float32)),
        "b1": np.ascontiguousarray(np.asarray(inputs["b1"], np.float32)),
        "b2": np.ascontiguousarray(np.asarray(inputs["b2"], np.float32)),
        "b3": np.ascontiguousarray(np.asarray(inputs["b3"], np.float32)),
    }
    in_maps = [
        {"p_in": P[c * B_LOC : (c + 1) * B_LOC], **shared} for c in range(N_CORES)
    ]
    res = run_bass_kernel_spmd(nc, in_maps, list(range(N_CORES)), trace=trace,
                               tmpdir=tmpdir)
    full = np.concatenate([res.results[c]["out"] for c in range(N_CORES)], axis=0)
    return full, res


def kernel(**inputs):
    full, _ = run(inputs)
    return full


# revision 6
# speedup vs baseline: 1.3365x; 1.3365x over previous
"""Trainium2 Bass kernel for the pairwise-score attention + gated MLP encoding.

Computation (per batch element b, p=1024 tokens, d=256 features):
    A[i,j]  = wa.P_i + wb.P_j + (P_i*wc).P_j
    itr     = softmax_j(A) @ P
    cat     = [P, itr]
    z       = tanh(cat@w1+b1); r = sigmoid(cat@w2+b2); f = sigmoid(cat@w3+b3)
    out     = r*P + f*z
Sharding: data-parallel over batch across 8 NeuronCores (4 batch el / core).

v3 design (fp8-DoubleRow scores/attention, bf16 MLP):
  - P loaded twice via gpsimd casting DMAs: natural bf16 (Pnb) and natural
    fp8e4 (Pn8).  P^T (bf16, PTb) via paired PE transposes (bf16, 1 cyc/row)
    evacuated by DVE.
  - Scores^T[j,i] (wa-term cancels in softmax; wb-term folded into the exp
    bias) via fp8e4 DoubleRow matmuls (contract 256/pass, 0.5 cyc/row):
    stationary PT8 (DVE cast of PTb), moving PcT8 = wc*PTb (DVE).
  - exp on ACT straight from 2-bank PSUM, fp8e4 output, sb bias (sb = P.wb
    via gpsimd mul + DVE row-reduce).
  - Softmax denominator (ones8 stationary) and itr^T numerator (Pn8 pair
    stationary) also fp8 DoubleRow over expST8.  DVE normalizes with
    reciprocal + mul into bf16 itrT.
  - MLP transposed in bf16 (out^T = (cat@w)^T), biases as ACT per-partition
    biases; sigmoid via 0.5+0.5*tanh(x/2) (single ACT table set).
  - Gating computes o' = 2*out in bf16 tensor_tensor ops (2x DVE rate); the
    0.5 rides the PSUM-evacuation tensor_scalar_mul after the bf16 output
    transposes (gpsimd).
  - MLP psum groups of batch b are interleaved with batch b+1's score
    matmuls at emission so the in-order ACT stream (6 MLP acts + 8 exps per
    batch ~ 15us, vs 17.4us of PE work) always has producers ahead of it and
    the PE never waits on exp at the attn phase boundary.
"""

import os
import sys

if "/opt/trn_rl_repo" not in sys.path:
    sys.path.insert(0, "/opt/trn_rl_repo")

import numpy as np

import concourse.bass as bass
import concourse.mybir as mybir
import concourse.tile as tile
from concourse import bacc
from concourse.bass_utils import run_bass_kernel_spmd
from concourse.masks import make_identity

F32 = mybir.dt.float32
BF16 = mybir.dt.bfloat16
FP8 = mybir.dt.float8e4
AF = mybir.ActivationFunctionType
ALU = mybir.AluOpType
AXX = mybir.AxisListType
DR = mybir.MatmulPerfMode.DoubleRow

B, PLEN, D = 32, 1024, 256
N_CORES = 8
B_LOC = B // N_CORES  # batch elements per core

NJ = PLEN // 128  # 8 token chunks of 128
ND = D // 128     # 2 feature chunks of 128
NPAIR = NJ // 2   # 4 token chunk-pairs (DoubleRow contraction)


def _emit(ctx, tc, P_in, w_att, w_mlp, b_mlp, out):
    nc = tc.nc
    ts = bass.ts
    ds = bass.ds

    const = ctx.enter_context(tc.tile_pool(name="const", bufs=1))
    pload = ctx.enter_context(tc.tile_pool(name="pload", bufs=3))
    ptp = ctx.enter_context(tc.tile_pool(name="ptp", bufs=2))
    pexp = ctx.enter_context(tc.tile_pool(name="pexp", bufs=2))
    pitr = ctx.enter_context(tc.tile_pool(name="pitr", bufs=2))
    pmlp = ctx.enter_context(tc.tile_pool(name="pmlp", bufs=2))
    pout = ctx.enter_context(tc.tile_pool(name="pout", bufs=2))
    ps_big = ctx.enter_context(tc.tile_pool(name="ps_big", bufs=3, space="PSUM"))
    ps_t2 = ctx.enter_context(tc.tile_pool(name="ps_t2", bufs=2, space="PSUM"))

    # ---- batch loads (emitted first so DMA starts immediately) ----
    def phase_load(b):
        pnb = pload.tile([128, NJ, 256], BF16, tag="pnb", name=f"pnb{b}")
        pn8 = pload.tile([128, NJ, 256], FP8, tag="pn8", name=f"pn8{b}")
        src = P_in[b, :, :].rearrange("(jc p) d -> p jc d", p=128)
        nc.gpsimd.dma_start(out=pnb, in_=src)
        nc.gpsimd.dma_start(out=pn8, in_=src)
        return pnb, pn8

    ld0 = phase_load(0)

    # ---- constants / weights (once per core) ----
    ident = const.tile([128, 128], F32)
    make_identity(nc, ident)
    identb = const.tile([128, 128], BF16)
    nc.vector.tensor_copy(out=identb, in_=ident)
    ones_f = const.tile([128, 256], F32)
    nc.vector.memset(ones_f, 1.0)
    ones8 = const.tile([128, 2, 128], FP8)
    nc.vector.tensor_copy(out=ones8, in_=ones_f.rearrange("p (t m) -> p t m", t=2))

    # wc per-partition scalars in transposed layout: [128,1] per d-chunk
    wc_sb = []
    for dc in range(ND):
        wc = const.tile([128, 1], F32, tag=f"wc{dc}")
        nc.gpsimd.dma_start(out=wc,
                            in_=w_att[ds(2 * D + dc * 128, 128)].unsqueeze(1))
        wc_sb.append(wc)
    # wb broadcast to all partitions and repeated per token-chunk:
    # [128, 8, 256] bf16 (for the sb row-reduction against natural-layout P)
    _wbs = w_att[ds(D, D)]
    wbb8 = const.tile([128, NJ, 256], BF16)
    nc.gpsimd.dma_start(
        out=wbb8,
        in_=bass.AP(tensor=_wbs.tensor, offset=_wbs.offset,
                    ap=[[0, 128], [0, NJ]] + list(_wbs.ap)),
    )

    # MLP weights: [512, 256] -> sbuf [128, 4(kc), 256] bf16 via casting DMA
    w_sb = []
    for wi in range(3):
        wt = const.tile([128, 4, D], BF16, tag=f"w{wi}")
        nc.gpsimd.dma_start(
            out=wt, in_=w_mlp[wi].rearrange("(kc k) d -> k kc d", k=128))
        w_sb.append(wt)

    # biases, per dout-chunk [128,1]; for r/f (sigmoid-via-tanh) we need b/2
    b_sb = []  # b_sb[wi][dc]
    for wi in range(3):
        chunks = []
        for dc in range(ND):
            bt = const.tile([128, 1], F32, tag=f"b{wi}{dc}")
            nc.gpsimd.dma_start(out=bt,
                                in_=b_mlp[wi][ds(dc * 128, 128)].unsqueeze(1))
            if wi > 0:
                bh = const.tile([128, 1], F32, tag=f"bh{wi}{dc}")
                nc.scalar.mul(out=bh, in_=bt, mul=0.5)
                bt = bh
            chunks.append(bt)
        b_sb.append(chunks)

    # ---- per-batch-element phases ----
    def phase_inT(b, pnb):
        # P^T (bf16) via paired PE transposes, DVE-evacuated
        ptb = ptp.tile([128, ND, PLEN], BF16, tag="ptb", name=f"ptb{b}")
        for dc in range(ND):
            for j2 in range(NJ // 2):
                pst = ps_t2.tile([128, 256], BF16, tag="pst", name="psti")
                nc.tensor.transpose(pst[:, 0:128], pnb[:, 2 * j2, ts(dc, 128)],
                                    identb)
                nc.tensor.transpose(pst[:, 128:256],
                                    pnb[:, 2 * j2 + 1, ts(dc, 128)], identb)
                nc.vector.tensor_copy(out=ptb[:, dc, ts(j2, 256)], in_=pst)
        return ptb

    def phase_prep(b, pnb, ptb):
        pt8 = ptp.tile([128, ND, PLEN], FP8, tag="pt8", name=f"pt8{b}")
        nc.vector.tensor_copy(out=pt8, in_=ptb)
        pct8 = ptp.tile([128, ND, PLEN], FP8, tag="pct8", name=f"pct8{b}")
        for dc in range(ND):
            nc.vector.tensor_scalar_mul(out=pct8[:, dc, :], in0=ptb[:, dc, :],
                                        scalar1=wc_sb[dc])
        # sb[j] = P_j . wb : gpsimd mul + one DVE row-reduce
        scr = ptp.tile([128, NJ, 256], BF16, tag="scr", name=f"scr{b}")
        nc.gpsimd.tensor_mul(out=scr, in0=pnb, in1=wbb8)
        sbc = ptp.tile([128, NJ], F32, tag="sbc", name=f"sbc{b}")
        nc.vector.reduce_sum(out=sbc, in_=scr, axis=AXX.X)
        return pt8, pct8, sbc

    def make_es(b):
        return [pexp.tile([128, 2, PLEN], FP8, tag=f"es{pr}", name=f"es{pr}_{b}")
                for pr in range(NPAIR)]

    def emit_score_jc(b, jc, pt8, pct8, sbc, es):
        pss = ps_big.tile([128, 1024], F32, tag="big", name=f"pss{jc}")
        for ic2 in range(2):
            nc.tensor.matmul(pss[:, ts(ic2, 512)],
                             pt8[:, :, ts(jc, 128)],
                             pct8[:, :, ts(ic2, 512)],
                             start=True, stop=True, perf_mode=DR)
        nc.scalar.activation(out=es[jc // 2][:, jc % 2, :], in_=pss,
                             func=AF.Exp, bias=sbc[:, ds(jc, 1)], scale=1.0)

    def phase_attn(b, pn8, es):
        # softmax denominators (replicated across partitions by ones8)
        psd = ps_big.tile([128, 1024], F32, tag="big", name="psd")
        for ic2 in range(2):
            for pr in range(NPAIR):
                nc.tensor.matmul(psd[:, ts(ic2, 512)], ones8,
                                 es[pr][:, :, ts(ic2, 512)],
                                 start=(pr == 0), stop=(pr == NPAIR - 1),
                                 perf_mode=DR)
        # itr^T numerator, directly in the layout the MLP consumes
        pits = []
        for dc in range(ND):
            pit = ps_big.tile([128, 1024], F32, tag="big", name=f"pit{dc}")
            for ic2 in range(2):
                for pr in range(NPAIR):
                    nc.tensor.matmul(pit[:, ts(ic2, 512)],
                                     pn8[:, ds(2 * pr, 2), ts(dc, 128)],
                                     es[pr][:, :, ts(ic2, 512)],
                                     start=(pr == 0), stop=(pr == NPAIR - 1),
                                     perf_mode=DR)
            pits.append(pit)
        return psd, pits

    def phase_norm(b, psd, pits):
        recipb = pitr.tile([128, PLEN], F32, tag="recipb", name=f"rec{b}")
        nc.vector.reciprocal_approx_fast(out=recipb, in_=psd)
        itrT = []
        for dc in range(ND):
            it = pitr.tile([128, PLEN], BF16, tag=f"it{dc}", name=f"itrT{dc}_{b}")
            nc.vector.tensor_mul(out=it, in0=pits[dc], in1=recipb)
            itrT.append(it)
        return itrT

    def emit_mlp_group(b, dc, wi, ptb, itrT):
        catT = [ptb[:, 0, :], ptb[:, 1, :], itrT[0], itrT[1]]
        psm = ps_big.tile([128, 1024], F32, tag="big", name=f"psm{dc}{wi}")
        for pc in range(2):
            for kc in range(4):
                nc.tensor.matmul(
                    psm[:, ts(pc, 512)],
                    w_sb[wi][:, kc, ts(dc, 128)],
                    catT[kc][:, ts(pc, 512)],
                    start=(kc == 0), stop=(kc == 3),
                )
        t = pmlp.tile([128, PLEN], BF16, tag=f"act{wi}", name=f"a{wi}d{dc}")
        if wi == 0:
            nc.scalar.activation(out=t, in_=psm, func=AF.Tanh,
                                 bias=b_sb[0][dc], scale=1.0)
        else:
            nc.scalar.activation(out=t, in_=psm, func=AF.Tanh,
                                 bias=b_sb[wi][dc], scale=0.5)
        return t

    def phase_gate(b, ptb, acts):
        # o' = 2*out = (t2+1)*P + (t3+1)*z, all-bf16 tensor_tensor (2x DVE)
        oT = []
        for dc in range(ND):
            z_t, t2, t3 = acts[dc]
            p_sl = ptb[:, dc, :]
            m1 = pmlp.tile([128, PLEN], BF16, tag="m1", name="m1", bufs=2)
            nc.vector.tensor_mul(out=m1, in0=t2, in1=p_sl)
            nc.vector.tensor_add(out=m1, in0=m1, in1=p_sl)
            m2 = pmlp.tile([128, PLEN], BF16, tag="m2", name="m2", bufs=2)
            nc.vector.tensor_mul(out=m2, in0=t3, in1=z_t)
            nc.vector.tensor_add(out=m2, in0=m2, in1=z_t)
            o = pmlp.tile([128, PLEN], BF16, tag=f"oT{dc}", name=f"oT{dc}_{b}")
            nc.vector.tensor_add(out=o, in0=m1, in1=m2)
            oT.append(o)
        return oT

    def phase_out(b, oT):
        for p2 in range(NJ):
            pst = ps_t2.tile([128, 256], BF16, tag="pst", name="psto")
            nc.tensor.transpose(pst[:, 0:128], oT[0][:, ts(p2, 128)], identb)
            nc.tensor.transpose(pst[:, 128:256], oT[1][:, ts(p2, 128)], identb)
            onat = pout.tile([128, D], F32, tag=f"on{p2}", name=f"onat{p2}")
            nc.vector.tensor_scalar_mul(out=onat, in0=pst, scalar1=0.5)
            nc.sync.dma_start(out=out[b, ts(p2, 128), :], in_=onat)

    # interleave pattern: scores jc's of b+1 between MLP psum groups of b
    MLP_ORDER = [(0, 0), (0, 1), (0, 2), (1, 0), (1, 1), (1, 2)]
    JC_BEFORE = [[0], [1], [2, 3], [4, 5], [6, 7], []]

    # ---- software-pipelined emission across batch elements ----
    lds = {0: ld0, 1: phase_load(1)}
    ptb0 = phase_inT(0, lds[0][0])
    pt8_0, pct8_0, sbc0 = phase_prep(0, lds[0][0], ptb0)
    es = make_es(0)
    for jc in range(NJ):
        emit_score_jc(0, jc, pt8_0, pct8_0, sbc0, es)
    cur = {"pnb": lds[0][0], "pn8": lds[0][1], "ptb": ptb0}
    oT_prev = None
    for b in range(B_LOC):
        last = b + 1 >= B_LOC
        if b + 2 < B_LOC:
            lds[b + 2] = phase_load(b + 2)
        psd, pits = phase_attn(b, cur["pn8"], es)
        itrT = phase_norm(b, psd, pits)
        if oT_prev is not None:
            phase_out(b - 1, oT_prev)
        if not last:
            nxt = {"pnb": lds[b + 1][0], "pn8": lds[b + 1][1]}
            nxt["ptb"] = phase_inT(b + 1, nxt["pnb"])
            pt8_n, pct8_n, sbc_n = phase_prep(b + 1, nxt["pnb"], nxt["ptb"])
            es_n = make_es(b + 1)
        # MLP of b interleaved with scores of b+1
        acts = [[None] * 3, [None] * 3]
        for gi, (dc, wi) in enumerate(MLP_ORDER):
            if not last:
                for jc in JC_BEFORE[gi]:
                    emit_score_jc(b + 1, jc, pt8_n, pct8_n, sbc_n, es_n)
            acts[dc][wi] = emit_mlp_group(b, dc, wi, cur["ptb"], itrT)
        oT = phase_gate(b, cur["ptb"], acts)
        oT_prev = oT
        if not last:
            es = es_n
            cur = nxt
    phase_out(B_LOC - 1, oT_prev)


_NC_CACHE = {}


def _build():
    if "nc" in _NC_CACHE:
        return _NC_CACHE["nc"]
    nc = bacc.Bacc("TRN2", target_bir_lowering=False, debug=False,
                   num_devices=N_CORES)
    P_in = nc.dram_tensor("p_in", [B_LOC, PLEN, D], F32, kind="ExternalInput").ap()
    w_att = nc.dram_tensor("w_att", [3 * D], F32, kind="ExternalInput").ap()
    w_mlp = [nc.dram_tensor(f"w{i}", [2 * D, D], F32, kind="ExternalInput").ap()
             for i in (1, 2, 3)]
    b_mlp = [nc.dram_tensor(f"b{i}", [D], F32, kind="ExternalInput").ap()
             for i in (1, 2, 3)]
    out = nc.dram_tensor("out", [B_LOC, PLEN, D], F32, kind="ExternalOutput").ap()

    from contextlib import ExitStack

    with tile.TileContext(nc) as tc, ExitStack() as ctx:
        _emit(ctx, tc, P_in, w_att, w_mlp, b_mlp, out)
    nc.compile()
    _NC_CACHE["nc"] = nc
    return nc


def run(inputs, trace=False, tmpdir=None):
    nc = _build()
    P = np.ascontiguousarray(np.asarray(inputs["P"], dtype=np.float32))
    shared = {
        "w_att": np.ascontiguousarray(np.asarray(inputs["w_itr_att"], np.float32)),
        "w1": np.ascontiguousarray(np.asarray(inputs["w1"], np.float32)),
        "w2": np.ascontiguousarray(np.asarray(inputs["w2"], np.float32)),
        "w3": np.ascontiguousarray(np.asarray(inputs["w3"], np.float32)),
        "b1": np.ascontiguousarray(np.asarray(inputs["b1"], np.float32)),
        "b2": np.ascontiguousarray(np.asarray(inputs["b2"], np.float32)),
        "b3": np.ascontiguousarray(np.asarray(inputs["b3"], np.float32)),
    }
    in_maps = [
        {"p_in": P[c * B_LOC : (c + 1) * B_LOC], **shared} for c in range(N_CORES)
    ]
    res = run_bass_kernel_spmd(nc, in_maps, list(range(N_CORES)), trace=trace,
                               tmpdir=tmpdir)
    full = np.concatenate([res.results[c]["out"] for c in range(N_CORES)], axis=0)
    return full, res


def kernel(**inputs):
    full, _ = run(inputs)
    return full
